# revision 20
# baseline (speedup 1.0000x reference)
"""Trainium2 Bass kernel for nn_AttentionBlock (B=4, C=256, N=4096).

Sharding: 8 cores = (batch b in 0..3) x (sequence half h in 0..1).

Math: with q = wq x + bq, k = wk x + bk, softmax over j is invariant to
per-i additive terms, so
    energy[i,j] ~ x_i^T A x_j + w_j,   A = wq^T wk,  w = (wk^T bq)^T x
(the bk and per-i terms drop out).  Each core computes, for its batch b
and its 2048 attention rows I:
    g = A^T-proj of xq            [C, 2048]  (bf16 matmul, fp8 store)
    vt_raw[j, :] = (wv x)^T       [4096, C]  col 257 = w_j (exp bias);
                                  vt = gamma*vt_raw (bf16), col 256 = 1
    sT[j, i] = sum_c x8[c,j] g8[c,i]          (fp8 DoubleRow, K=256, 1 MM)
    p = exp(sT + (w_j - 60))                  (fixed-shift softmax, bf16)
    vaT[i, :] = sum_j p[j,i] * vt[j, :]   -> gamma*numerator | denominator
    outT[i, d] = (xT[i,d] + gamma*bv[d]) + vaT[i, d] / vaT[i, C]
Host reassembles out[b][:, I] = outT.T.  No collectives needed.

Precision: x and wv stream in fp8-e4m3 (TRN float8e4, max 240 -- pose
values are ~N(0,1), no clipping needed); the energy matmul runs
DoubleRow fp8 (K=256 in one MM at ~2x bf16 rate), as does the
v-projection.  g is computed from bf16 inputs in fp32 PSUM then stored
fp8 in the paired [p, c_sub, i] layout DoubleRow wants.  exp and the
p*V matmul stay bf16 with fp32 PSUM accumulation.  gamma is folded into
vt at the vproj copy (per-partition tensor_scalar) and gamma*bv is
pre-computed host-side, so the epilogue is reciprocal + one fused
scalar_tensor_tensor per 128-row tile.

Fixed shift: energies are sums of 256 ~N(0,1) products (std ~19, row max
in [43,127] here), so exp(e-60) stays within fp32/bf16 range both ways.

DMA: one queue per dma_start and ~200ns/queue of end-of-kernel teardown,
so starts are consolidated (~20 queues).  Critical-path order: xq0 ahead
of wv on the scalar queue (gproj chunk 0 gates the first S stage); at
ahead of x_od on gpsimd; aux+xt on the otherwise-idle vector queue so
x_od chunks are never stuck behind them.  The first i-block's attention
streams behind the x chunks; i-blocks 1-3 run back-to-back from SBUF.
PE warms up on scratch matmuls during the initial DMA wait (HAM ramp).
PSUM: 4 s-tiles + 4 va accumulators = 8 banks exactly.
"""

import sys

sys.path.insert(0, "/opt/trn_rl_repo")

import ml_dtypes
import numpy as np

import concourse.bass as bass
import concourse.mybir as mybir
import concourse.tile as tile
from concourse import bacc
from concourse.bass_utils import run_bass_kernel_spmd

B, C, N = 4, 256, 4096
NCORES = 8
HALF = N // 2  # attention rows per core
P = 128
F32 = mybir.dt.float32
BF16 = mybir.dt.bfloat16
F8 = mybir.dt.float8e4
SHIFT = 60.0
EXP = mybir.ActivationFunctionType.Exp
ADD = mybir.AluOpType.add
MULT = mybir.AluOpType.mult
DR = mybir.MatmulPerfMode.DoubleRow
CP = C + 4  # V^T cols: [0:C]=V, C=ones (denom), C+1=w (exp bias), rest pad
WCOL = C + 1
AUXW = 264  # aux: [0:C]=gamma*bv, C..: [1.0? no] -- [C]=gamma, rest pad
NWARM = 5  # PE warmup matmuls during initial DMA wait (p-state ramp)
NST = N // P  # 32 j-tile stages per i-block


def _bcast_ap(handle_ap, parts=P):
    """Partition-broadcast a DRAM AP (stride-0 partition dim) for DMA."""
    return bass.AP(
        tensor=handle_ap.tensor,
        offset=handle_ap.offset,
        ap=[[0, parts]] + list(handle_ap.ap),
    )


def build_nc():
    nc = bacc.Bacc("TRN2", target_bir_lowering=False)

    x_ext = nc.declare_dram_parameter("x8", [C, N], F8, isOutput=False)
    xq_ext = nc.declare_dram_parameter("xq", [C, HALF], BF16, isOutput=False)
    xt_ext = nc.declare_dram_parameter("xt", [HALF, C], BF16, isOutput=False)
    at_ext = nc.declare_dram_parameter("atT", [C, C], BF16, isOutput=False)
    wv_ext = nc.declare_dram_parameter("wvT", [C, CP], F8, isOutput=False)
    aux_ext = nc.declare_dram_parameter("aux", [AUXW], F32, isOutput=False)
    out_ext = nc.declare_dram_parameter("out_t", [HALF, C], BF16, isOutput=True)

    x_v = x_ext[:, :].rearrange("(s p) n -> p s n", p=P)
    xq_v = xq_ext[:, :].rearrange("(s p) n -> p s n", p=P)
    xt_v = xt_ext[:, :].rearrange("(t p) c -> p t c", p=P)
    out_v = out_ext[:, :].rearrange("(t p) c -> p t c", p=P)
    at_v = at_ext[:, :].rearrange("(s p) d -> p s d", p=P)
    wv_v = wv_ext[:, :].rearrange("(s p) d -> p s d", p=P)

    with tile.TileContext(nc) as tc:
        with (
            tc.tile_pool(name="xin", bufs=1) as xin,
            tc.tile_pool(name="big", bufs=1) as big,
            tc.tile_pool(name="wp", bufs=1) as wp,
            tc.tile_pool(name="small", bufs=1) as small,
            tc.tile_pool(name="expp", bufs=5) as expp,
            tc.tile_pool(name="epi", bufs=8) as epi,
            tc.tile_pool(name="outp", bufs=2) as outp,
            tc.tile_pool(name="spsum", bufs=1, space="PSUM") as spsum,
            tc.tile_pool(name="vapsum", bufs=4, space="PSUM") as vapsum,
        ):
            # Two persistent 2-bank PSUM tiles; their four 512-col halves
            # rotate like the old 4-buf pool for gproj/vproj/block-0 S tiles,
            # and blocks 1-3 exp a full [P,1024] pair in ONE activation
            # (biasless: w rides vt, see below) -- halves the ACT op count.
            sp0 = spsum.tile([P, 1024], F32, name="sp0")
            sp1 = spsum.tile([P, 1024], F32, name="sp1")
            _sph = [sp0[:, :512], sp0[:, 512:], sp1[:, :512], sp1[:, 512:]]
            _rot = [0]

            def sp_half():
                h = _sph[_rot[0] % 4]
                _rot[0] += 1
                return h

            # ---- PE warmup on scratch zeros (p-state ramp during DMA wait) --
            scratch = wp.tile([P, 512], BF16)
            nc.vector.memset(scratch, 0.0)
            for _ in range(NWARM):
                nc.tensor.matmul(sp_half(), lhsT=scratch[:, :P], rhs=scratch)

            # ---- DMA (one hardware queue per start; keep starts few and the
            # critical ones first on each engine queue) ----
            at_sb = wp.tile([P, 2, C], BF16)
            wv_sb = wp.tile([P, 2, CP], F8)
            x_ev = xin.tile([P, 2, N // 2], F8)
            x_od = xin.tile([P, 2, N // 2], F8)
            xq_sb = xin.tile([P, 2, HALF], BF16)
            xt_sb = xin.tile([P, HALF // P, C], BF16)
            aux_sb = small.tile([P, AUXW], F32)
            # gpsimd queue: at (gates gproj), aux (gates vproj copy), odd x
            nc.gpsimd.dma_start(out=at_sb, in_=at_v)
            nc.gpsimd.dma_start(out=aux_sb, in_=_bcast_ap(aux_ext[:]))
            for ch in (1, 3, 5, 7):
                dsl = slice((ch // 2) * 512, (ch // 2) * 512 + 512)
                nc.gpsimd.dma_start(out=x_od[:, :, dsl], in_=x_v[:, :, ch * 512 : (ch + 1) * 512])
            # scalar queue: xq chunk 0 (gates gproj), wv, xq rest, xt (late)
            nc.scalar.dma_start(out=xq_sb[:, :, :512], in_=xq_v[:, :, :512])
            nc.scalar.dma_start(out=wv_sb, in_=wv_v)
            nc.scalar.dma_start(out=xq_sb[:, :, 512:], in_=xq_v[:, :, 512:])
            nc.scalar.dma_start(out=xt_sb, in_=xt_v)
            # sync queue: even x chunks, first chunk split for earliest start
            nc.sync.dma_start(out=x_ev[:, :, :256], in_=x_v[:, :, :256])
            nc.sync.dma_start(out=x_ev[:, :, 256:512], in_=x_v[:, :, 256:512])
            for ch in (2, 4, 6):
                dsl = slice((ch // 2) * 512, (ch // 2) * 512 + 512)
                nc.sync.dma_start(out=x_ev[:, :, dsl], in_=x_v[:, :, ch * 512 : (ch + 1) * 512])

            def xj8(jt):
                # DoubleRow lhsT j-tile of x: [128, 2, 128] paired c layout
                c, q = divmod(jt, 4)
                t = x_ev if c % 2 == 0 else x_od
                o = (c // 2) * 512 + q * P
                return t[:, :, o : o + P]

            g_sb = big.tile([P, 2, HALF], F8)
            vt_sb = big.tile([P, NST, CP], BF16)
            w_sb = big.tile([P, NST], F32)  # per-j exp bias: w_j - SHIFT
            shift_sb = small.tile([P, 2], F32)  # [-SHIFT, +SHIFT] act biases
            nc.vector.memset(shift_sb[:, 0:1], -SHIFT)
            nc.vector.memset(shift_sb[:, 1:2], SHIFT)
            # denominator ones column (copies below never touch col C)
            nc.vector.memset(vt_sb[:, :, C:CP], 0.0)
            nc.vector.memset(vt_sb[:, :, C : C + 1], 1.0)
            gam_sb = aux_sb[:, C : C + 1]

            # ---- streamed attention ----
            # PSUM->SBUF copies all ride DVE: the Act engine does nothing but
            # exp during attention (it is the near-critical engine per stage)

            def copy_dve(dst, src):
                nc.vector.tensor_scalar_add(dst, src, 0.0)

            def gproj_chunk(c):  # 512 i-columns of G = (wq^T wk)^T-proj of xq
                sl = slice(c * 512, (c + 1) * 512)
                for d_sub in range(2):
                    ps = sp_half()
                    for c_sub in range(2):
                        nc.tensor.matmul(
                            ps,
                            lhsT=at_sb[:, c_sub, d_sub * P : (d_sub + 1) * P],
                            rhs=xq_sb[:, c_sub, sl],
                            start=(c_sub == 0),
                            stop=(c_sub == 1),
                        )
                    copy_dve(g_sb[:, d_sub, sl], ps)

            def vproj_tile(jt):  # one 128-row j-tile: V block + w column
                ps = sp_half()
                nc.tensor.matmul(
                    ps[:, : WCOL + 1],
                    lhsT=xj8(jt),
                    rhs=wv_sb[:, :, : WCOL + 1],
                    perf_mode=DR,
                )
                # w copy first: E(jt) waits only on this short op, not on the
                # full vt copy (shortens the vproj->exp latency chain)
                nc.vector.tensor_scalar_add(
                    w_sb[:, jt : jt + 1], ps[:, WCOL : WCOL + 1], -SHIFT
                )
                # vt = gamma * (x^T wv) + gamma*bv: since sum_j attn = 1, the
                # +gamma*bv lands in the numerator as den*gbva and the final
                # division restores out = gamma*va + gamma*bv + xT with a raw
                # (un-pre-biased) residual tile -- no xtb pass needed.
                nc.vector.scalar_tensor_tensor(
                    vt_sb[:, jt, :C],
                    in0=ps[:, :C],
                    scalar=gam_sb,
                    in1=aux_sb[:, :C],
                    op0=MULT,
                    op1=ADD,
                )

            def make_stages(ib, va_ps):
                # block 0: per-tile exp with per-j bias w (vt not yet e^w
                # scaled); blocks 1-3 use the paired biasless variant below
                isl = slice(ib * 512, (ib + 1) * 512)
                s_tiles = {}
                e_tiles = {}

                def stage_S(k):
                    ps = sp_half()
                    nc.tensor.matmul(
                        ps, lhsT=xj8(k), rhs=g_sb[:, :, isl], perf_mode=DR
                    )
                    s_tiles[k] = ps

                def stage_E(k):
                    e = expp.tile([P, 512], BF16, tag="e")
                    nc.scalar.activation(
                        e, s_tiles.pop(k), EXP, bias=w_sb[:, k : k + 1]
                    )
                    e_tiles[k] = e

                def stage_V(k):
                    e = e_tiles.pop(k)
                    for i_sub in range(4):
                        nc.tensor.matmul(
                            va_ps[i_sub][:, : WCOL + 1],
                            lhsT=e[:, i_sub * P : (i_sub + 1) * P],
                            rhs=vt_sb[:, k, : WCOL + 1],
                            start=(k == 0),
                            stop=(k == NST - 1),
                            skip_group_check=True,
                        )

                return stage_S, stage_E, stage_V

            def make_stages_paired(ib, va_ps):
                # blocks 1-3: S(k even)/S(k odd) land in the two halves of one
                # 2-bank psum tile; ONE biasless exp covers the [P,1024] pair
                # (w folded into vt via the e^w scale applied during block 1)
                isl = slice(ib * 512, (ib + 1) * 512)
                e2_tiles = {}

                def stage_S(k):
                    nc.tensor.matmul(
                        sp_half(), lhsT=xj8(k), rhs=g_sb[:, :, isl], perf_mode=DR
                    )

                def stage_E2(k):  # k odd: exp the (k-1, k) pair
                    t = expp.tile([P, 1024], BF16, tag="e2")
                    src = sp0 if ((k // 2) % 2 == 0) else sp1
                    nc.scalar.activation(t, src, EXP, bias=shift_sb[:, 0:1])
                    e2_tiles[k // 2] = t

                def stage_V(k):
                    t = e2_tiles[k // 2] if k % 2 == 0 else e2_tiles.pop(k // 2)
                    off = (k % 2) * 512
                    for i_sub in range(4):
                        nc.tensor.matmul(
                            va_ps[i_sub][:, : WCOL + 1],
                            lhsT=t[:, off + i_sub * P : off + (i_sub + 1) * P],
                            rhs=vt_sb[:, k, : WCOL + 1],
                            start=(k == 0),
                            stop=(k == NST - 1),
                            skip_group_check=True,
                        )

                return stage_S, stage_E2, stage_V

            def epilogue(ib, va_ps):
                o_sb = outp.tile([P, 4, C], BF16)
                for i_sub in range(4):
                    rec = epi.tile([P, 1], F32, tag="rec")
                    nc.vector.reciprocal(rec, va_ps[i_sub][:, C : C + 1])
                    t = ib * 4 + i_sub
                    # out = (gamma*num) * (1/den) + (xT + gamma*bv), fused
                    nc.vector.scalar_tensor_tensor(
                        o_sb[:, i_sub, :],
                        in0=va_ps[i_sub][:, :C],
                        scalar=rec,
                        in1=xt_sb[:, t, :],
                        op0=MULT,
                        op1=ADD,
                    )
                    if ib == 3 and i_sub == 1:
                        nc.sync.dma_start(
                            out=out_v[:, ib * 4 : ib * 4 + 2, :], in_=o_sb[:, :2, :]
                        )
                if ib == 3:
                    nc.sync.dma_start(
                        out=out_v[:, ib * 4 + 2 : ib * 4 + 4, :], in_=o_sb[:, 2:, :]
                    )
                else:
                    nc.sync.dma_start(
                        out=out_v[:, ib * 4 : ib * 4 + 4, :], in_=o_sb
                    )

            # i-block 0 streams behind the x chunks
            va_ps0 = [
                vapsum.tile([P, CP], F32, tag="vaps", name=f"va_ps_0_{t}")
                for t in range(4)
            ]
            S0, E0, V0 = make_stages(0, va_ps0)

            gproj_chunk(0)
            for c in range(8):
                for jt in range(4 * c, 4 * c + 4):
                    vproj_tile(jt)
                    S0(jt)
                    E0(jt)
                    if jt >= 3:
                        V0(jt - 3)
            for ic in range(1, 4):
                gproj_chunk(ic)  # G slices for i-blocks 1-3
            # e^w per j (w_sb holds w-SHIFT, so bias back by +SHIFT); blocks
            # 1-3 fold the per-j softmax bias into vt instead of the exp
            ew_sb = small.tile([P, NST], F32)
            nc.scalar.activation(ew_sb, w_sb, EXP, bias=shift_sb[:, 1:2])
            V0(NST - 3)
            V0(NST - 2)
            V0(NST - 1)
            epilogue(0, va_ps0)

            # i-blocks 1-3 from SBUF
            for ib in range(1, 4):
                va_ps = [
                    vapsum.tile([P, CP], F32, tag="vaps", name=f"va_ps_{ib}_{t}")
                    for t in range(4)
                ]
                while _rot[0] % 4:  # align S pairs to tile boundaries
                    _rot[0] += 1
                S, E2, V = make_stages_paired(ib, va_ps)
                for k in range(NST):
                    S(k)
                    if ib == 1:
                        # scale vt rows (v cols + ones/denominator col) by
                        # e^{w_j}, after block 0's V consumed the raw tile
                        nc.vector.tensor_scalar_mul(
                            vt_sb[:, k, : C + 1],
                            vt_sb[:, k, : C + 1],
                            ew_sb[:, k : k + 1],
                        )
                    if k % 2 == 1:
                        E2(k)
                    if k >= 4:
                        V(k - 4)
                for k in range(NST - 4, NST):
                    V(k)
                epilogue(ib, va_ps)

    nc.finalize()
    return nc


def make_in_maps(pose_f, wq, bq, wk, bk, wv, bv, gamma):
    bf = ml_dtypes.bfloat16
    f8 = ml_dtypes.float8_e4m3
    pose_f = np.asarray(pose_f, dtype=np.float32)
    wq = np.asarray(wq, np.float32)
    wk = np.asarray(wk, np.float32)
    wv = np.asarray(wv, np.float32)
    bq = np.asarray(bq, np.float32)
    gam = float(np.asarray(gamma, np.float32)[0])
    # energy = x^T (wq^T wk) x + (wk^T bq)^T x  (bk/per-i terms drop in softmax)
    atT = np.ascontiguousarray((wq.T @ wk).astype(bf))
    beta = wk.T @ bq  # [C]
    wvT = np.zeros((C, CP), np.float32)
    wvT[:, :C] = wv.T
    wvT[:, WCOL] = beta
    wvT = np.ascontiguousarray(wvT.astype(f8))
    aux = np.zeros(AUXW, np.float32)
    aux[:C] = gam * np.asarray(bv, np.float32)
    aux[C] = gam
    pose_bf = pose_f.astype(bf)
    pose_f8 = pose_f.astype(f8)
    in_maps = []
    for c in range(NCORES):
        b, h = divmod(c, 2)
        sl = slice(h * HALF, (h + 1) * HALF)
        in_maps.append(
            {
                "x8": pose_f8[b],
                "xq": np.ascontiguousarray(pose_bf[b][:, sl]),
                "xt": np.ascontiguousarray(pose_bf[b][:, sl].T),
                "atT": atT,
                "wvT": wvT,
                "aux": aux,
            }
        )
    return in_maps


def assemble(results):
    out = np.empty((B, C, N), np.float32)
    for c in range(NCORES):
        b, h = divmod(c, 2)
        out[b, :, h * HALF : (h + 1) * HALF] = results[c]["out_t"].T.astype(np.float32)
    return out


_NC_CACHE = []


def run(in_maps, **kwargs):
    if not _NC_CACHE:
        _NC_CACHE.append(build_nc())
    return run_bass_kernel_spmd(
        _NC_CACHE[0], in_maps, core_ids=list(range(NCORES)), **kwargs
    )


def kernel(**inputs):
    in_maps = make_in_maps(**inputs)
    res = run(in_maps)
    return assemble(res.results)


# revision 21
# speedup vs baseline: 1.1489x; 1.1489x over previous
"""Trainium2 Bass kernel for nn_AttentionBlock (B=4, C=256, N=4096).

Sharding: 8 cores = (batch b in 0..3) x (sequence half h in 0..1).

Math: with q = wq x + bq, k = wk x + bk, softmax over j is invariant to
per-i additive terms, so
    energy[i,j] ~ x_i^T A x_j + w_j,   A = wq^T wk,  w = (wk^T bq)^T x
(the bk and per-i terms drop out).  Each core computes, for its batch b
and its 2048 attention rows I:
    g = A^T-proj of xq            [C, 2048]  (bf16 matmul, fp8 store)
    vt_raw[j, :] = (wv x)^T       [4096, C]  col 257 = w_j (exp bias);
                                  vt = gamma*vt_raw (bf16), col 256 = 1
    sT[j, i] = sum_c x8[c,j] g8[c,i]          (fp8 DoubleRow, K=256, 1 MM)
    p = exp(sT + (w_j - 60))                  (fixed-shift softmax, bf16)
    vaT[i, :] = sum_j p[j,i] * vt[j, :]   -> gamma*numerator | denominator
    outT[i, d] = (xT[i,d] + gamma*bv[d]) + vaT[i, d] / vaT[i, C]
Host reassembles out[b][:, I] = outT.T.  No collectives needed.

Precision: x and wv stream in fp8-e4m3 (TRN float8e4, max 240 -- pose
values are ~N(0,1), no clipping needed); the energy matmul runs
DoubleRow fp8 (K=256 in one MM at ~2x bf16 rate), as does the
v-projection.  g is computed from bf16 inputs in fp32 PSUM then stored
fp8 in the paired [p, c_sub, i] layout DoubleRow wants.  exp and the
p*V matmul stay bf16 with fp32 PSUM accumulation.  gamma is folded into
vt at the vproj copy (per-partition tensor_scalar) and gamma*bv is
pre-computed host-side, so the epilogue is reciprocal + one fused
scalar_tensor_tensor per 128-row tile.

Fixed shift: energies are sums of 256 ~N(0,1) products (std ~19, row max
in [43,127] here), so exp(e-60) stays within fp32/bf16 range both ways.

DMA: one queue per dma_start and ~200ns/queue of end-of-kernel teardown,
so starts are consolidated (~20 queues).  Critical-path order: xq0 ahead
of wv on the scalar queue (gproj chunk 0 gates the first S stage); at
ahead of x_od on gpsimd; aux+xt on the otherwise-idle vector queue so
x_od chunks are never stuck behind them.  The first i-block's attention
streams behind the x chunks; i-blocks 1-3 run back-to-back from SBUF.
PE warms up on scratch matmuls during the initial DMA wait (HAM ramp).
PSUM: 4 s-tiles + 4 va accumulators = 8 banks exactly.
"""

import sys

sys.path.insert(0, "/opt/trn_rl_repo")

import ml_dtypes
import numpy as np

import concourse.bass as bass
import concourse.mybir as mybir
import concourse.tile as tile
from concourse import bacc
from concourse.bass_utils import run_bass_kernel_spmd

B, C, N = 4, 256, 4096
NCORES = 8
HALF = N // 2  # attention rows per core
P = 128
F32 = mybir.dt.float32
BF16 = mybir.dt.bfloat16
F8 = mybir.dt.float8e4
SHIFT = 60.0
EXP = mybir.ActivationFunctionType.Exp
ADD = mybir.AluOpType.add
MULT = mybir.AluOpType.mult
DR = mybir.MatmulPerfMode.DoubleRow
CP = C + 4  # V^T cols: [0:C]=V, C=ones (denom), C+1=w (exp bias), rest pad
WCOL = C + 1
AUXW = 264  # aux: [0:C]=gamma*bv, C..: [1.0? no] -- [C]=gamma, rest pad
NWARM = 5  # PE warmup matmuls during initial DMA wait (p-state ramp)
NST = N // P  # 32 j-tile stages per i-block


def _bcast_ap(handle_ap, parts=P):
    """Partition-broadcast a DRAM AP (stride-0 partition dim) for DMA."""
    return bass.AP(
        tensor=handle_ap.tensor,
        offset=handle_ap.offset,
        ap=[[0, parts]] + list(handle_ap.ap),
    )


def build_nc():
    nc = bacc.Bacc("TRN2", target_bir_lowering=False)

    x_ext = nc.declare_dram_parameter("x8", [C, N], F8, isOutput=False)
    xq_ext = nc.declare_dram_parameter("xq", [C, HALF], BF16, isOutput=False)
    xt_ext = nc.declare_dram_parameter("xt", [HALF, C], BF16, isOutput=False)
    at_ext = nc.declare_dram_parameter("atT", [C, C], BF16, isOutput=False)
    wv_ext = nc.declare_dram_parameter("wvT", [C, CP], F8, isOutput=False)
    aux_ext = nc.declare_dram_parameter("aux", [AUXW], F32, isOutput=False)
    out_ext = nc.declare_dram_parameter("out_t", [HALF, C], BF16, isOutput=True)

    x_v = x_ext[:, :].rearrange("(s p) n -> p s n", p=P)
    xq_v = xq_ext[:, :].rearrange("(s p) n -> p s n", p=P)
    xt_v = xt_ext[:, :].rearrange("(t p) c -> p t c", p=P)
    out_v = out_ext[:, :].rearrange("(t p) c -> p t c", p=P)
    at_v = at_ext[:, :].rearrange("(s p) d -> p s d", p=P)
    wv_v = wv_ext[:, :].rearrange("(s p) d -> p s d", p=P)

    with tile.TileContext(nc) as tc:
        with (
            tc.tile_pool(name="xin", bufs=1) as xin,
            tc.tile_pool(name="big", bufs=1) as big,
            tc.tile_pool(name="wp", bufs=1) as wp,
            tc.tile_pool(name="small", bufs=1) as small,
            tc.tile_pool(name="expp", bufs=5) as expp,
            tc.tile_pool(name="epi", bufs=8) as epi,
            tc.tile_pool(name="outp", bufs=2) as outp,
            tc.tile_pool(name="spsum", bufs=4, space="PSUM") as spsum,
            tc.tile_pool(name="vapsum", bufs=4, space="PSUM") as vapsum,
        ):
            # ---- PE warmup on scratch zeros (p-state ramp during DMA wait) --
            scratch = wp.tile([P, 512], BF16)
            nc.vector.memset(scratch, 0.0)
            for _ in range(NWARM):
                ps = spsum.tile([P, 512], F32, tag="spsum")
                nc.tensor.matmul(ps, lhsT=scratch[:, :P], rhs=scratch)

            # ---- DMA (one hardware queue per start; keep starts few and the
            # critical ones first on each engine queue) ----
            at_sb = wp.tile([P, 2, C], BF16)
            wv_sb = wp.tile([P, 2, CP], F8)
            x_ev = xin.tile([P, 2, N // 2], F8)
            x_od = xin.tile([P, 2, N // 2], F8)
            xq_sb = xin.tile([P, 2, HALF], BF16)
            xt_sb = xin.tile([P, HALF // P, C], BF16)
            aux_sb = small.tile([P, AUXW], F32)
            # gpsimd queue: at (gates gproj), aux (gates vproj copy), odd x
            nc.gpsimd.dma_start(out=at_sb, in_=at_v)
            nc.gpsimd.dma_start(out=aux_sb, in_=_bcast_ap(aux_ext[:]))
            for ch in (1, 3, 5, 7):
                dsl = slice((ch // 2) * 512, (ch // 2) * 512 + 512)
                nc.gpsimd.dma_start(out=x_od[:, :, dsl], in_=x_v[:, :, ch * 512 : (ch + 1) * 512])
            # scalar queue: xq chunk 0 (gates gproj), wv, xq rest, xt (late)
            nc.scalar.dma_start(out=xq_sb[:, :, :512], in_=xq_v[:, :, :512])
            nc.scalar.dma_start(out=wv_sb, in_=wv_v)
            nc.scalar.dma_start(out=xq_sb[:, :, 512:], in_=xq_v[:, :, 512:])
            nc.scalar.dma_start(out=xt_sb, in_=xt_v)
            # sync queue: even x chunks, first chunk split for earliest start
            nc.sync.dma_start(out=x_ev[:, :, :256], in_=x_v[:, :, :256])
            nc.sync.dma_start(out=x_ev[:, :, 256:512], in_=x_v[:, :, 256:512])
            for ch in (2, 4, 6):
                dsl = slice((ch // 2) * 512, (ch // 2) * 512 + 512)
                nc.sync.dma_start(out=x_ev[:, :, dsl], in_=x_v[:, :, ch * 512 : (ch + 1) * 512])

            def xj8(jt):
                # DoubleRow lhsT j-tile of x: [128, 2, 128] paired c layout
                c, q = divmod(jt, 4)
                t = x_ev if c % 2 == 0 else x_od
                o = (c // 2) * 512 + q * P
                return t[:, :, o : o + P]

            g_sb = big.tile([P, 2, HALF], F8)
            vt_sb = big.tile([P, NST, CP], BF16)
            w_sb = big.tile([P, NST], F32)  # per-j exp bias: w_j - SHIFT
            # denominator ones column (copies below never touch col C)
            nc.vector.memset(vt_sb[:, :, C:CP], 0.0)
            nc.vector.memset(vt_sb[:, :, C : C + 1], 1.0)
            gam_sb = aux_sb[:, C : C + 1]

            # ---- streamed attention ----
            # PSUM->SBUF copies all ride DVE: the Act engine does nothing but
            # exp during attention (it is the near-critical engine per stage)

            def copy_dve(dst, src):
                nc.vector.tensor_scalar_add(dst, src, 0.0)

            def gproj_part(sl):  # i-columns sl of G = (wq^T wk)^T-proj of xq
                n = sl.stop - sl.start
                for d_sub in range(2):
                    ps = spsum.tile([P, 512], F32, tag="spsum")
                    for c_sub in range(2):
                        nc.tensor.matmul(
                            ps[:, :n],
                            lhsT=at_sb[:, c_sub, d_sub * P : (d_sub + 1) * P],
                            rhs=xq_sb[:, c_sub, sl],
                            start=(c_sub == 0),
                            stop=(c_sub == 1),
                        )
                    copy_dve(g_sb[:, d_sub, sl], ps[:, :n])

            def gproj_chunk(c):
                gproj_part(slice(c * 512, (c + 1) * 512))

            def vproj_tile(jt):  # one 128-row j-tile: V block + w column
                ps = spsum.tile([P, 512], F32, tag="spsum")
                nc.tensor.matmul(
                    ps[:, : WCOL + 1],
                    lhsT=xj8(jt),
                    rhs=wv_sb[:, :, : WCOL + 1],
                    perf_mode=DR,
                )
                # w copy first: E(jt) waits only on this short op, not on the
                # full vt copy (shortens the vproj->exp latency chain)
                nc.vector.tensor_scalar_add(
                    w_sb[:, jt : jt + 1], ps[:, WCOL : WCOL + 1], -SHIFT
                )
                # vt = gamma * (x^T wv) + gamma*bv: since sum_j attn = 1, the
                # +gamma*bv lands in the numerator as den*gbva and the final
                # division restores out = gamma*va + gamma*bv + xT with a raw
                # (un-pre-biased) residual tile -- no xtb pass needed.
                nc.vector.scalar_tensor_tensor(
                    vt_sb[:, jt, :C],
                    in0=ps[:, :C],
                    scalar=gam_sb,
                    in1=aux_sb[:, :C],
                    op0=MULT,
                    op1=ADD,
                )

            def make_stages(ib, va_ps):
                isl = slice(ib * 512, (ib + 1) * 512)
                s_tiles = {}
                e_tiles = {}

                def stage_S(k):
                    ps = spsum.tile([P, 512], F32, tag="spsum")
                    nc.tensor.matmul(
                        ps, lhsT=xj8(k), rhs=g_sb[:, :, isl], perf_mode=DR
                    )
                    s_tiles[k] = ps

                def stage_E(k):
                    e = expp.tile([P, 512], BF16, tag="e")
                    nc.scalar.activation(
                        e, s_tiles.pop(k), EXP, bias=w_sb[:, k : k + 1]
                    )
                    e_tiles[k] = e

                def stage_V(k):
                    e = e_tiles.pop(k)
                    for i_sub in range(4):
                        nc.tensor.matmul(
                            va_ps[i_sub][:, : WCOL + 1],
                            lhsT=e[:, i_sub * P : (i_sub + 1) * P],
                            rhs=vt_sb[:, k, : WCOL + 1],
                            start=(k == 0),
                            stop=(k == NST - 1),
                            skip_group_check=True,
                        )

                return stage_S, stage_E, stage_V

            def epilogue(ib, va_ps):
                o_sb = outp.tile([P, 4, C], BF16)
                for i_sub in range(4):
                    rec = epi.tile([P, 1], F32, tag="rec")
                    nc.vector.reciprocal(rec, va_ps[i_sub][:, C : C + 1])
                    t = ib * 4 + i_sub
                    # out = (gamma*num) * (1/den) + (xT + gamma*bv), fused
                    nc.vector.scalar_tensor_tensor(
                        o_sb[:, i_sub, :],
                        in0=va_ps[i_sub][:, :C],
                        scalar=rec,
                        in1=xt_sb[:, t, :],
                        op0=MULT,
                        op1=ADD,
                    )
                    if ib == 3 and i_sub == 1:
                        nc.sync.dma_start(
                            out=out_v[:, ib * 4 : ib * 4 + 2, :], in_=o_sb[:, :2, :]
                        )
                if ib == 3:
                    nc.sync.dma_start(
                        out=out_v[:, ib * 4 + 2 : ib * 4 + 4, :], in_=o_sb[:, 2:, :]
                    )
                else:
                    nc.sync.dma_start(
                        out=out_v[:, ib * 4 : ib * 4 + 4, :], in_=o_sb
                    )

            # i-block 0 streams behind the x chunks
            va_ps0 = [
                vapsum.tile([P, CP], F32, tag="vaps", name=f"va_ps_0_{t}")
                for t in range(4)
            ]
            S0, E0, V0 = make_stages(0, va_ps0)

            gproj_chunk(0)
            for c in range(8):
                for jt in range(4 * c, 4 * c + 4):
                    vproj_tile(jt)
                    S0(jt)
                    E0(jt)
                    if jt >= 3:
                        V0(jt - 3)
            for ic in range(1, 4):
                gproj_chunk(ic)  # G slices for i-blocks 1-3
            V0(NST - 3)
            V0(NST - 2)
            V0(NST - 1)
            epilogue(0, va_ps0)

            # i-blocks 1-3 from SBUF
            for ib in range(1, 4):
                va_ps = [
                    vapsum.tile([P, CP], F32, tag="vaps", name=f"va_ps_{ib}_{t}")
                    for t in range(4)
                ]
                S, E, V = make_stages(ib, va_ps)
                for k in range(NST):
                    S(k)
                    E(k)
                    if k >= 3:
                        V(k - 3)
                V(NST - 3)
                V(NST - 2)
                V(NST - 1)
                epilogue(ib, va_ps)

    nc.finalize()
    return nc


def make_in_maps(pose_f, wq, bq, wk, bk, wv, bv, gamma):
    bf = ml_dtypes.bfloat16
    f8 = ml_dtypes.float8_e4m3
    pose_f = np.asarray(pose_f, dtype=np.float32)
    wq = np.asarray(wq, np.float32)
    wk = np.asarray(wk, np.float32)
    wv = np.asarray(wv, np.float32)
    bq = np.asarray(bq, np.float32)
    gam = float(np.asarray(gamma, np.float32)[0])
    # energy = x^T (wq^T wk) x + (wk^T bq)^T x  (bk/per-i terms drop in softmax)
    atT = np.ascontiguousarray((wq.T @ wk).astype(bf))
    beta = wk.T @ bq  # [C]
    wvT = np.zeros((C, CP), np.float32)
    wvT[:, :C] = wv.T
    wvT[:, WCOL] = beta
    wvT = np.ascontiguousarray(wvT.astype(f8))
    aux = np.zeros(AUXW, np.float32)
    aux[:C] = gam * np.asarray(bv, np.float32)
    aux[C] = gam
    pose_bf = pose_f.astype(bf)
    pose_f8 = pose_f.astype(f8)
    in_maps = []
    for c in range(NCORES):
        b, h = divmod(c, 2)
        sl = slice(h * HALF, (h + 1) * HALF)
        in_maps.append(
            {
                "x8": pose_f8[b],
                "xq": np.ascontiguousarray(pose_bf[b][:, sl]),
                "xt": np.ascontiguousarray(pose_bf[b][:, sl].T),
                "atT": atT,
                "wvT": wvT,
                "aux": aux,
            }
        )
    return in_maps


def assemble(results):
    out = np.empty((B, C, N), np.float32)
    for c in range(NCORES):
        b, h = divmod(c, 2)
        out[b, :, h * HALF : (h + 1) * HALF] = results[c]["out_t"].T.astype(np.float32)
    return out


_NC_CACHE = []


def run(in_maps, **kwargs):
    if not _NC_CACHE:
        _NC_CACHE.append(build_nc())
    return run_bass_kernel_spmd(
        _NC_CACHE[0], in_maps, core_ids=list(range(NCORES)), **kwargs
    )


def kernel(**inputs):
    in_maps = make_in_maps(**inputs)
    res = run(in_maps)
    return assemble(res.results)
